# revision 11
# baseline (speedup 1.0000x reference)
"""Trainium2 Bass kernel for nn_BaseGenerator_71451075936296.

6-layer post-norm dense transformer (B=32, S=256, E=1024, H=16, F=4096,
V=192) with a per-head additive attention bias gathered from distance /
isopen embedding tables.

Strategy: data-parallel over batch across 8 NeuronCores (4 sequences =
1024 tokens per core), weights replicated. All GEMMs run in bf16 on the
TensorEngine with fp32 PSUM accumulation.

v2 redesign vs the first working kernel:
- Transposed-softmax attention: scores^T = K^T Q computed per 128-key
  tile (two heads packed concurrently via PE row-tiling), bias added
  with one identity matmul, single Exp per head, attn@V in token-major
  form with a ones-column in V providing the softmax denominator, then
  a per-partition normalize.  No P-transposes, no accum reads, no
  vector-engine softmax normalize.
- LayerNorm: residual added into PSUM by the PE (identity matmul),
  bn_stats reads PSUM directly, rstd computed on the vector engine via
  Newton-rsqrt with a bit-trick seed (no Sqrt activation-table loads),
  normalize is one scalar-engine op with per-partition scale/bias.
- All 128x128 transposes are regular matmuls (lhsT=x chunk, rhs=I),
  ~2.5x faster than transpose-mode.
- Only two activation-table loads per layer (exp set <-> gelu set).
- Zero-bias / trivial-LN build variants picked from the actual inputs.
"""

import math
from contextlib import ExitStack

import numpy as np
import ml_dtypes

import concourse.bass as bass
import concourse.mybir as mybir
import concourse.tile as tile
from concourse import bacc
from concourse.bass_utils import run_bass_kernel_spmd
from concourse.masks import make_identity

B, S, E, H, F, L, V = 32, 256, 1024, 16, 4096, 6, 192
DH = E // H          # 64
NCORES = 8
BL = B // NCORES     # 4 sequences per core
T = BL * S           # 1024 tokens per core
P = 128
NT = T // P          # 8 token tiles
NE = E // P          # 8 E chunks
NF = F // P          # 32 F chunks
EPS = 1e-5
NEG = -1e30

bf16 = mybir.dt.bfloat16
f32 = mybir.dt.float32
i32 = mybir.dt.int32
AF = mybir.ActivationFunctionType
OP = mybir.AluOpType

nbf16 = ml_dtypes.bfloat16

GELU_FUNC = AF.Gelu
RSQRT_MAGIC = 0x5F3759DF


def _emit(ctx, tc, d, layers, zero_bias, ln_trivial):
    nc = tc.nc

    pool = lambda name, bufs, **kw: ctx.enter_context(
        tc.tile_pool(name=name, bufs=bufs, **kw))

    const = pool("const", 1)
    ident = const.tile([P, P], bf16)
    make_identity(nc, ident)
    ones_row = const.tile([1, P], bf16)
    nc.vector.memset(ones_row, 1.0)

    # ---- persistent state tiles ----
    big = pool("big", 1)
    x_t = [big.tile([P, E], bf16, tag=f"x{t}", name=f"x{t}") for t in range(NT)]
    # xT in two halves (token halves) for fine-grained deps
    xT = [big.tile([P, NE, T // 2], bf16, tag=f"xT{h}", name=f"xT{h}")
          for h in range(2)]
    # ov: time-shared q/k (qk+attention) then FFN hidden h (per th half)
    ov = [big.tile([P, 2, 512], bf16, tag=f"ov{m}", name=f"ov{m}")
          for m in range(16)]
    # v with a ones column per head block: [P, H, 65]
    v65 = [big.tile([P, H, 65], bf16, tag=f"v{t}", name=f"v{t}")
           for t in range(NT)]
    # attention output (token-major) and its transpose (feature-major, per b)
    o_t = [big.tile([P, E], bf16, tag=f"o{t}", name=f"o{t}") for t in range(NT)]
    aoT = [big.tile([P, NE, 2 * P], bf16, tag=f"aoT{b}", name=f"aoT{b}")
           for b in range(BL)]

    ps = pool("ps", 8, space="PSUM")
    wp = pool("wp", 3)        # [P,1024]bf16 all-K weight stripes (qk / w1)
    wsp = pool("wsp", 3)      # [P,1024]bf16 streamed rhs stripes (v / wo)
    w2p = pool("w2p", 3)      # [P,1024]bf16 w2 stripes
    wgp = pool("wgp", 8)      # [P,V]bf16 logit stripes
    bp = pool("bp", 6)        # biasT tiles [P,2,384]bf16 per unit
    ep = pool("ep", 6)        # exp tiles [P,2,384]bf16 per unit
    tmp = pool("tmp", 2)      # f32 [P,V] logits staging
    st = pool("st", 24)       # small stats
    lnp = pool("lnp", 2)      # replicated ln vecs [P,E]
    colp = pool("colp", 2)    # per-layer bias column tiles
    rowp = pool("rowp", 2)    # [1,E] bias rows
    xnp = pool("xnp", 2)      # general-LN scratch [P,512]

    def psum(shape, dt=f32):
        return ps.tile(shape, dt, tag="ps", name="ps")

    def dma(out, in_):
        nc.sync.dma_start(out=out, in_=in_)

    def row_ap(ap1d):
        return ap1d.rearrange("(o e) -> o e", o=1)

    # ones columns of v65 (written once; v writes never touch col 64)
    for t in range(NT):
        nc.vector.memset(v65[t][:, :, 64:65], 1.0)

    # ---------------- layernorm helpers ----------------
    def rsqrt_newton(ve):
        """rstd [P,1] f32 from var+eps tile ve, on the vector engine."""
        ib = st.tile([P, 1], i32, tag="ib", name="ib")
        nc.vector.tensor_scalar(
            out=ib, in0=ve.bitcast(i32), scalar1=1, scalar2=None,
            op0=OP.logical_shift_right)
        y0i = st.tile([P, 1], i32, tag="y0i", name="y0i")
        nc.vector.tensor_scalar(
            out=y0i, in0=ib, scalar1=-1, scalar2=RSQRT_MAGIC,
            op0=OP.mult, op1=OP.add)
        y = y0i.bitcast(f32)
        for it in range(2):
            a = st.tile([P, 1], f32, tag=f"nr{it}a", name="nra")
            nc.vector.tensor_tensor(out=a, in0=y, in1=y, op=OP.mult)
            b_ = st.tile([P, 1], f32, tag=f"nr{it}b", name="nrb")
            nc.vector.scalar_tensor_tensor(
                out=b_, in0=a, scalar=-0.5, in1=ve, op0=OP.mult, op1=OP.mult)
            y2 = st.tile([P, 1], f32, tag=f"nr{it}y", name="nry")
            nc.vector.scalar_tensor_tensor(
                out=y2, in0=b_, scalar=1.5, in1=y, op0=OP.add, op1=OP.mult)
            y = y2
        return y

    def ln_from_psum(tt, halves, ln_rep):
        """x_t[tt] = LN(psum halves) (+scale/bias). halves: two [P,512] f32
        PSUM APs holding x + sublayer(x) already summed."""
        stats = st.tile([P, 2, 6], f32, tag="bnst", name="bnst")
        for sg in range(2):
            nc.vector.bn_stats(out=stats[:, sg, :], in_=halves[sg])
        mv = st.tile([P, 2], f32, tag="bnmv", name="bnmv")
        nc.vector.bn_aggr(out=mv, in_=stats)
        ve = st.tile([P, 1], f32, tag="ve", name="ve")
        nc.vector.tensor_scalar_add(out=ve, in0=mv[:, 1:2], scalar1=EPS)
        rstd = rsqrt_newton(ve)
        nmr = st.tile([P, 1], f32, tag="nmr", name="nmr")
        nc.vector.scalar_tensor_tensor(
            out=nmr, in0=mv[:, 0:1], scalar=-1.0, in1=rstd,
            op0=OP.mult, op1=OP.mult)
        if ln_trivial:
            for sg in range(2):
                nc.scalar.activation(
                    out=x_t[tt][:, sg * 512:(sg + 1) * 512], in_=halves[sg],
                    func=AF.Identity, bias=nmr, scale=rstd)
        else:
            s_rep, b_rep = ln_rep
            for sg in range(2):
                xn = xnp.tile([P, 512], f32, tag="xn", name="xn")
                nc.scalar.activation(
                    out=xn, in_=halves[sg],
                    func=AF.Identity, bias=nmr, scale=rstd)
                xs = xnp.tile([P, 512], f32, tag="xs2", name="xs2")
                nc.vector.tensor_tensor(
                    out=xs, in0=xn, in1=s_rep[:, sg * 512:(sg + 1) * 512],
                    op=OP.mult)
                nc.vector.tensor_tensor(
                    out=x_t[tt][:, sg * 512:(sg + 1) * 512], in0=xs,
                    in1=b_rep[:, sg * 512:(sg + 1) * 512], op=OP.add)

    def load_ln(s_ap, b_ap):
        if ln_trivial:
            return None
        s_rep = lnp.tile([P, E], bf16, tag="lns", name="lns")
        b_rep = lnp.tile([P, E], bf16, tag="lnb", name="lnb")
        nc.gpsimd.dma_start(out=s_rep, in_=s_ap.to_broadcast([P, E]))
        nc.gpsimd.dma_start(out=b_rep, in_=b_ap.to_broadcast([P, E]))
        return s_rep, b_rep

    # ---------------- transpose helper (regular matmuls) ----------------
    def transpose_tile(src, dst_ap_fn):
        """Transpose [P, E] src into feature-major dst.
        dst_ap_fn(g) -> [P, 4, P] destination AP for e-chunk group g."""
        for g in range(2):
            pt = psum([P, 4, P])
            for e4 in range(4):
                e = g * 4 + e4
                nc.tensor.matmul(
                    out=pt[:, e4, :], lhsT=src[:, e * P:(e + 1) * P],
                    rhs=ident, start=True, stop=True)
            nc.vector.tensor_copy(out=dst_ap_fn(g), in_=pt)

    def x_to_xT(tt):
        th, c = tt // 4, tt % 4
        transpose_tile(
            x_t[tt],
            lambda g: xT[th][:, g * 4:(g + 1) * 4, c * P:(c + 1) * P])

    # ---- embedding (tok gather + pos add precomputed host-side) ----
    for t in range(NT):
        dma(x_t[t], d["x0"].ap()[t * P:(t + 1) * P, :])
        x_to_xT(t)

    # ---------------- layers ----------------
    for l in range(layers):
        if not zero_bias:
            bqk_c = colp.tile([P, 16], f32, tag="bqk", name="bqkc")
            dma(bqk_c, d["bqk"].ap()[l])
            bv_row = rowp.tile([1, E], bf16, tag="rowv", name="bvrow")
            dma(bv_row, row_ap(d["bv_row"].ap()[l]))
            b1_c = colp.tile([P, NF], f32, tag="b1", name="b1c")
            dma(b1_c, d["b1"].ap()[l])
            bo_row = rowp.tile([1, E], bf16, tag="row", name="borow")
            dma(bo_row, row_ap(d["bo"].ap()[l]))
            b2_row = rowp.tile([1, E], bf16, tag="row2", name="b2row")
            dma(b2_row, row_ap(d["b2"].ap()[l]))

        # --- q,k projections: feature-major, weights stationary ---
        for m in range(16):
            wt = wp.tile([P, NE * P], bf16, tag="wp", name="wqk")
            dma(wt, d["wqk"].ap()[l, m])
            for hf in range(2):
                pse = psum([P, 512])
                for k in range(NE):
                    nc.tensor.matmul(
                        out=pse, lhsT=wt[:, k * P:(k + 1) * P],
                        rhs=xT[hf][:, k, :],
                        start=(k == 0), stop=(k == NE - 1))
                if zero_bias:
                    nc.scalar.activation(
                        out=ov[m][:, hf, :], in_=pse, func=AF.Identity)
                else:
                    nc.scalar.activation(
                        out=ov[m][:, hf, :], in_=pse,
                        func=AF.Identity, bias=bqk_c[:, m:m + 1])

        # --- v projection: token-major, into v65 (tq groups of 2 tiles) ---
        for tq in range(4):
            pse = [[psum([P, 512]) for _ in range(2)] for _ in range(2)]
            for k in range(NE):
                wv = wsp.tile([P, E], bf16, tag="wsp", name="wv")
                dma(wv, d["wv"].ap()[l, k])
                for t2 in range(2):
                    tt = tq * 2 + t2
                    for hf in range(2):
                        nc.tensor.matmul(
                            out=pse[t2][hf],
                            lhsT=xT[tt // 4][:, k, (tt % 4) * P:(tt % 4 + 1) * P],
                            rhs=wv[:, hf * 512:(hf + 1) * 512],
                            start=(k == 0),
                            stop=(zero_bias and k == NE - 1))
            for t2 in range(2):
                tt = tq * 2 + t2
                for hf in range(2):
                    if not zero_bias:
                        nc.tensor.matmul(
                            out=pse[t2][hf], lhsT=ones_row,
                            rhs=bv_row[:, hf * 512:(hf + 1) * 512],
                            start=False, stop=True)
                    src = pse[t2][hf].rearrange("p (h d) -> p h d", h=8)
                    nc.vector.tensor_copy(
                        out=v65[tt][:, hf * 8:(hf + 1) * 8, 0:64], in_=src)

        # --- attention + O-transpose + Wo + LN1, pipelined per b ---
        ln1 = load_ln(row_ap(d["ln1s"].ap()[l]), row_ap(d["ln1b"].ap()[l]))

        def qk_ap(m, r, c0, w):
            hf, off = divmod(c0, 512)
            return ov[m][r:r + DH, hf, off:off + w]

        for b in range(BL):
            for jj in range(H // 2):
                bias2 = bp.tile([P, 2, 384], bf16, tag="bp", name="bias2")
                dma(bias2, d["biasT"].ap()[b, :, 2 * jj:2 * jj + 2, :])
                e2 = ep.tile([P, 2, 384], bf16, tag="ep", name="e2")
                ps_av = psum([P, 4, 65])
                for hh in range(2):
                    r = hh * DH
                    sc = psum([P, 384])
                    # scores^T: kb0 tile (q 0:256), kb1 tile (q 128:256)
                    nc.tensor.matmul(
                        out=sc[:, 0:256],
                        lhsT=qk_ap(8 + jj, r, b * 256, P),
                        rhs=qk_ap(jj, r, b * 256, 256),
                        start=True, stop=False)
                    nc.tensor.matmul(
                        out=sc[:, 256:384],
                        lhsT=qk_ap(8 + jj, r, b * 256 + P, P),
                        rhs=qk_ap(jj, r, b * 256 + P, P),
                        start=False, stop=False, skip_group_check=True)
                    nc.tensor.matmul(
                        out=sc, lhsT=ident, rhs=bias2[:, hh, :],
                        start=False, stop=True, skip_group_check=True)
                    nc.scalar.activation(out=e2[:, hh, :], in_=sc, func=AF.Exp)
                    # attn @ V with ones-column denominator
                    h_ = 2 * jj + hh
                    nc.tensor.matmul(
                        out=ps_av[:, hh * 2, :], lhsT=e2[:, hh, 0:P],
                        rhs=v65[2 * b][:, h_, :], start=True, stop=True)
                    nc.tensor.matmul(
                        out=ps_av[:, hh * 2 + 1, :], lhsT=e2[:, hh, P:256],
                        rhs=v65[2 * b][:, h_, :], start=True, stop=False)
                    nc.tensor.matmul(
                        out=ps_av[:, hh * 2 + 1, :], lhsT=e2[:, hh, 256:384],
                        rhs=v65[2 * b + 1][:, h_, :], start=False, stop=True)
                rinv = st.tile([P, 4], f32, tag="rinv", name="rinv")
                nc.vector.reciprocal(out=rinv, in_=ps_av[:, :, 64])
                for hh in range(2):
                    h_ = 2 * jj + hh
                    for qt in range(2):
                        nc.vector.tensor_scalar_mul(
                            out=o_t[2 * b + qt][:, h_ * DH:(h_ + 1) * DH],
                            in0=ps_av[:, hh * 2 + qt, 0:64],
                            scalar1=rinv[:, hh * 2 + qt:hh * 2 + qt + 1])

            # O transpose for this b (2 token tiles)
            for qt in range(2):
                transpose_tile(
                    o_t[2 * b + qt],
                    lambda g, _qt=qt: aoT[b][:, g * 4:(g + 1) * 4,
                                             _qt * P:(_qt + 1) * P])

            # Wo for this b's two token tiles
            pse = [[psum([P, 512]) for _ in range(2)] for _ in range(2)]
            for c in range(NE):
                wo = wsp.tile([P, E], bf16, tag="wsp", name="wo")
                dma(wo, d["wo"].ap()[l, c])
                for qt in range(2):
                    for hf in range(2):
                        nc.tensor.matmul(
                            out=pse[qt][hf],
                            lhsT=aoT[b][:, c, qt * P:(qt + 1) * P],
                            rhs=wo[:, hf * 512:(hf + 1) * 512],
                            start=(c == 0), stop=False)
            for qt in range(2):
                tt = 2 * b + qt
                for hf in range(2):
                    if not zero_bias:
                        nc.tensor.matmul(
                            out=pse[qt][hf], lhsT=ones_row,
                            rhs=bo_row[:, hf * 512:(hf + 1) * 512],
                            start=False, stop=False)
                    nc.tensor.matmul(
                        out=pse[qt][hf], lhsT=ident,
                        rhs=x_t[tt][:, hf * 512:(hf + 1) * 512],
                        start=False, stop=True)
                ln_from_psum(tt, pse[qt], ln1)
                x_to_xT(tt)

        # --- FFN (th halves) ---
        ln2 = load_ln(row_ap(d["ln2s"].ap()[l]), row_ap(d["ln2b"].ap()[l]))
        for th in range(2):
            for m in range(NF):
                wt = wp.tile([P, NE * P], bf16, tag="wp", name="w1t")
                dma(wt, d["w1"].ap()[l, m])
                ph = psum([P, 512])
                for k in range(NE):
                    nc.tensor.matmul(
                        out=ph, lhsT=wt[:, k * P:(k + 1) * P],
                        rhs=xT[th][:, k, :],
                        start=(k == 0), stop=(k == NE - 1))
                if zero_bias:
                    nc.scalar.activation(out=ov[m // 2][:, m % 2, :], in_=ph,
                                         func=GELU_FUNC)
                else:
                    nc.scalar.activation(out=ov[m // 2][:, m % 2, :], in_=ph,
                                         func=GELU_FUNC,
                                         bias=b1_c[:, m:m + 1])
            pys = [[psum([P, 512]) for _ in range(2)] for _ in range(4)]
            for k in range(NF):
                w2s = w2p.tile([P, E], bf16, tag="w2p", name="w2s")
                dma(w2s, d["w2"].ap()[l, k])
                for t4 in range(4):
                    for eh in range(2):
                        nc.tensor.matmul(
                            out=pys[t4][eh],
                            lhsT=ov[k // 2][:, k % 2, t4 * P:(t4 + 1) * P],
                            rhs=w2s[:, eh * 512:(eh + 1) * 512],
                            start=(k == 0), stop=False)
            for t4 in range(4):
                tt = th * 4 + t4
                for eh in range(2):
                    if not zero_bias:
                        nc.tensor.matmul(
                            out=pys[t4][eh], lhsT=ones_row,
                            rhs=b2_row[:, eh * 512:(eh + 1) * 512],
                            start=False, stop=False)
                    nc.tensor.matmul(
                        out=pys[t4][eh], lhsT=ident,
                        rhs=x_t[tt][:, eh * 512:(eh + 1) * 512],
                        start=False, stop=True)
                ln_from_psum(tt, pys[t4], ln2)
                x_to_xT(tt)

    # ---------------- final LN + logits ----------------
    lnf = load_ln(row_ap(d["lnfs"].ap()), row_ap(d["lnfb"].ap()))
    wgs = []
    for k in range(NE):
        wg = wgp.tile([P, V], bf16, tag="wg", name="wg")
        dma(wg, d["wg"].ap()[k])
        wgs.append(wg)
    if not zero_bias:
        bg_row = rowp.tile([1, V], bf16, tag="rowg", name="bgrow")
        dma(bg_row, row_ap(d["bg"].ap()))
    for t in range(NT):
        # final LN from SBUF (no residual): copy halves into PSUM via PE
        pse = [psum([P, 512]) for _ in range(2)]
        for sg in range(2):
            nc.tensor.matmul(
                out=pse[sg], lhsT=ident,
                rhs=x_t[t][:, sg * 512:(sg + 1) * 512],
                start=True, stop=True)
        ln_from_psum(t, pse, lnf)
        x_to_xT(t)
    for t in range(NT):
        th, c = t // 4, t % 4
        pl = psum([P, V])
        for k in range(NE):
            nc.tensor.matmul(
                out=pl, lhsT=xT[th][:, k, c * P:(c + 1) * P], rhs=wgs[k],
                start=(k == 0), stop=(zero_bias and k == NE - 1))
        if not zero_bias:
            nc.tensor.matmul(out=pl, lhsT=ones_row, rhs=bg_row,
                             start=False, stop=True)
        lo = tmp.tile([P, V], f32, tag="lo", name="lo")
        nc.any.tensor_copy(out=lo, in_=pl)
        dma(d["out"].ap()[t * P:(t + 1) * P, :], lo)


def _declare(nc):
    d = {}
    def inp(name, shape, dt):
        d[name] = nc.dram_tensor(name, list(shape), dt, kind="ExternalInput")
    inp("x0", [T, E], bf16)
    inp("biasT", [BL, P, H, 384], bf16)
    inp("wqk", [L, 16, P, NE * P], bf16)
    inp("wv", [L, NE, P, E], bf16)
    inp("bqk", [L, P, 16], f32)
    inp("bv_row", [L, E], bf16)
    inp("wo", [L, NE, P, E], bf16)
    inp("bo", [L, E], bf16)
    inp("w1", [L, NF, P, NE * P], bf16)
    inp("b1", [L, P, NF], f32)
    inp("w2", [L, NF, P, E], bf16)
    inp("b2", [L, E], bf16)
    inp("ln1s", [L, E], bf16)
    inp("ln1b", [L, E], bf16)
    inp("ln2s", [L, E], bf16)
    inp("ln2b", [L, E], bf16)
    inp("lnfs", [E], bf16)
    inp("lnfb", [E], bf16)
    inp("wg", [NE, P, V], bf16)
    inp("bg", [V], bf16)
    d["out"] = nc.dram_tensor("out", [T, V], f32, kind="ExternalOutput")
    return d


_BUILT = {}


def build(layers=L, zero_bias=True, ln_trivial=True):
    key = ("nc", layers, str(GELU_FUNC), zero_bias, ln_trivial)
    if key in _BUILT:
        return _BUILT[key]
    nc = bacc.Bacc("TRN2", target_bir_lowering=False, debug=False)
    d = _declare(nc)
    with tile.TileContext(nc) as tc:
        with ExitStack() as ctx:
            _emit(ctx, tc, d, layers, zero_bias, ln_trivial)
    nc.compile()
    _BUILT[key] = nc
    return nc


def prep_shared(inputs):
    g = lambda k: np.asarray(inputs[k])
    sh = {}

    WqkvT = np.ascontiguousarray(g("Wqkv").transpose(0, 2, 1)).astype(np.float32)  # [L,E,3E]
    qkw = WqkvT[:, :, :2 * E].copy()
    qkw[:, :, :E] *= 0.125          # fold 1/sqrt(DH) into q weights
    qk = qkw.reshape(L, NE, P, 16, P).transpose(0, 3, 2, 1, 4)
    sh["wqk"] = np.ascontiguousarray(qk.reshape(L, 16, P, NE * P)).astype(nbf16)
    sh["wv"] = np.ascontiguousarray(WqkvT[:, :, 2 * E:].reshape(L, NE, P, E)).astype(nbf16)
    bqkv = g("bqkv").astype(np.float32)
    bqk = bqkv[:, :2 * E].copy()
    bqk[:, :E] *= 0.125
    sh["bqk"] = np.ascontiguousarray(bqk.reshape(L, 16, P).transpose(0, 2, 1))
    sh["bv_row"] = np.ascontiguousarray(bqkv[:, 2 * E:]).astype(nbf16)

    WoT = g("Wo").transpose(0, 2, 1)
    sh["wo"] = np.ascontiguousarray(WoT.reshape(L, NE, P, E)).astype(nbf16)
    sh["bo"] = g("bo").astype(nbf16)

    W1T = g("W1").transpose(0, 2, 1)  # [L,E,F]
    w1 = W1T.reshape(L, NE, P, NF, P).transpose(0, 3, 2, 1, 4)
    sh["w1"] = np.ascontiguousarray(w1.reshape(L, NF, P, NE * P)).astype(nbf16)
    sh["b1"] = np.ascontiguousarray(
        g("b1").astype(np.float32).reshape(L, NF, P).transpose(0, 2, 1))

    W2T = g("W2").transpose(0, 2, 1)  # [L,F,E]
    sh["w2"] = np.ascontiguousarray(W2T.reshape(L, NF, P, E)).astype(nbf16)
    sh["b2"] = g("b2").astype(nbf16)

    for ks, kd in [("ln1_s", "ln1s"), ("ln1_b", "ln1b"),
                   ("ln2_s", "ln2s"), ("ln2_b", "ln2b")]:
        sh[kd] = g(ks).astype(nbf16)
    sh["lnfs"] = g("lnf_s").astype(nbf16)
    sh["lnfb"] = g("lnf_b").astype(nbf16)

    WgT = np.asarray(g("Wg")).T  # [E,V]
    sh["wg"] = np.ascontiguousarray(WgT.reshape(NE, P, V)).astype(nbf16)
    sh["bg"] = g("bg").astype(nbf16)
    return sh


def prep_biasT(inputs):
    """[B,S,S,H] gathered bias -> causal-packed transposed layout
    [B, 128(key-in-tile), H, 384] bf16:
      cols   0:256 = kb0 keys (0:128)   x q 0:256
      cols 256:384 = kb1 keys (128:256) x q 128:256
    """
    dist = np.asarray(inputs["distance_squares"]).astype(np.int64)
    isop = np.asarray(inputs["isopen_squares"]).astype(np.int64)
    de = np.asarray(inputs["dist_emb"]).astype(np.float32)[dist]    # [B,S,S,H]
    ie = np.asarray(inputs["isopen_emb"]).astype(np.float32)[isop]  # [B,S,S,H]
    bias = de + ie                                                  # [B,q,k,H]
    causal = np.tril(np.ones((S, S), bool))                         # q >= k
    bias = np.where(causal[None, :, :, None], bias, NEG)
    pad_id = int(np.asarray(inputs["pad_id"]))
    kpm = np.asarray(inputs["sequences"]) == pad_id                 # [B,S]
    bias = np.where(kpm[:, None, :, None], NEG, bias)
    biasT = bias.transpose(0, 2, 3, 1)                              # [B,k,H,q]
    out = np.full((B, P, H, 384), NEG, np.float32)
    out[:, :, :, 0:256] = biasT[:, 0:128, :, :]
    out[:, :, :, 256:384] = biasT[:, 128:256, :, 128:256]
    return np.ascontiguousarray(out).astype(nbf16)


def make_in_maps(inputs):
    sh = prep_shared(inputs)
    biasT = prep_biasT(inputs)
    seq = np.asarray(inputs["sequences"])
    x0 = (np.asarray(inputs["tok_emb"]).astype(np.float32)[seq] * math.sqrt(E)
          + np.asarray(inputs["pos_emb"]).astype(np.float32)[None])  # [B,S,E]
    x0 = x0.astype(nbf16)
    in_maps = []
    for c in range(NCORES):
        m = dict(sh)
        m["x0"] = np.ascontiguousarray(
            x0[c * BL:(c + 1) * BL].reshape(T, E))
        m["biasT"] = np.ascontiguousarray(biasT[c * BL:(c + 1) * BL])
        in_maps.append(m)
    return in_maps


LAST_RES = None


def kernel(**inputs):
    global LAST_RES
    g = lambda k: np.asarray(inputs[k])
    zero_bias = all(
        not np.any(g(k)) for k in ("bqkv", "bo", "b1", "b2", "bg"))
    ln_trivial = (
        np.all(g("ln1_s") == 1) and np.all(g("ln2_s") == 1)
        and np.all(g("lnf_s") == 1)
        and not np.any(g("ln1_b")) and not np.any(g("ln2_b"))
        and not np.any(g("lnf_b")))
    nc = build(zero_bias=zero_bias, ln_trivial=ln_trivial)
    in_maps = make_in_maps(inputs)
    res = run_bass_kernel_spmd(nc, in_maps, core_ids=list(range(NCORES)))
    LAST_RES = res
    out = np.concatenate(
        [np.asarray(r["out"]).reshape(BL, S, V) for r in res.results], axis=0)
    return out.astype(np.float32)
